# revision 1
# baseline (speedup 1.0000x reference)
import numpy as np
import jax
import jax.numpy as jnp
from jax.sharding import Mesh, PartitionSpec as P
from jax.experimental.shard_map import shard_map

# nn_Attention: windowed attention (Swin-style) with relative position bias
# and residual score. Full shapes: x[1024,49,768], prev[1024,12,49,49],
# qkv_w[768,2304], proj_w[768,768], proj_b[768], bias_table[169,12],
# rel_idx[49,49] int32. Data-parallel over B across 8 cores (128 windows/core).

NUM_HEADS = 12
N_CORES = 8

_compiled = None


def _attn_block(x, prev, qkv_w, proj_w, proj_b, bias):
    B, N, C = x.shape
    H = NUM_HEADS
    hd = C // H
    scale = hd ** -0.5
    qkv = (x @ qkv_w).reshape(B, N, 3, H, hd)
    q = qkv[:, :, 0].transpose(0, 2, 1, 3)  # [B,H,N,hd]
    k = qkv[:, :, 1].transpose(0, 2, 1, 3)
    v = qkv[:, :, 2].transpose(0, 2, 1, 3)
    attn_score = jnp.einsum('bhnd,bhmd->bhnm', q, k) * scale
    attn_score = attn_score + bias[None]
    attn_score = attn_score + prev
    prev_out = attn_score
    attn = jax.nn.softmax(attn_score, axis=-1)
    out = jnp.einsum('bhnm,bhmd->bhnd', attn, v).transpose(0, 2, 1, 3).reshape(B, N, C)
    out = out @ proj_w + proj_b
    return out, prev_out


def _get_compiled():
    global _compiled
    if _compiled is None:
        devices = np.array(jax.devices()[:N_CORES])
        mesh = Mesh(devices, ('b',))
        fn = shard_map(
            _attn_block,
            mesh=mesh,
            in_specs=(P('b'), P('b'), P(), P(), P(), P()),
            out_specs=(P('b'), P('b')),
        )
        _compiled = jax.jit(fn)
    return _compiled


def kernel(x, prev, qkv_w, proj_w, proj_b, bias_table, rel_idx):
    N = x.shape[1]
    # tiny gather done on host: bias [H,N,N]
    bias = np.asarray(bias_table)[np.asarray(rel_idx).reshape(-1)]
    bias = bias.reshape(N, N, NUM_HEADS).transpose(2, 0, 1).copy()

    fn = _get_compiled()
    out, prev_out = fn(
        jnp.asarray(x, dtype=jnp.float32),
        jnp.asarray(prev, dtype=jnp.float32),
        jnp.asarray(qkv_w, dtype=jnp.float32),
        jnp.asarray(proj_w, dtype=jnp.float32),
        jnp.asarray(proj_b, dtype=jnp.float32),
        jnp.asarray(bias, dtype=jnp.float32),
    )
    return np.asarray(out), np.asarray(prev_out)
